# revision 13
# baseline (speedup 1.0000x reference)
"""Linear attention (elu(x)+1 feature map) Bass/Tile kernel for Trainium2.

Problem: B=4, H=16, S=4096, D=64, fp32.
  Qf = elu(Q)+1; Kf = (elu(K)+1)*mask
  KV = einsum('bhsd,bhse->bhde', Kf, V); Ksum = sum_s Kf*mask
  out = (Qf @ KV) / (Qf . Ksum)

Sharding: 64 (b,h) pairs data-parallel over 8 cores, 8 pairs each. No
collectives.

Per-core v2 design (DMA-bandwidth bound):
  Layout s = 32*p + j (p = SBUF partition, j = 0..31): every Q/K/V/O
  transfer is a single fully-contiguous 1 MB DMA with 8 KB per-partition
  lines (vs 256B lines in v1).
  Per pair:
    - PE transposes raw fp32 Q in 16 [128,128] blocks -> tp PSUM.
    - Scalar ACT computes Exp/Relu of tp into bf16 SBUF; one fused DVE
      scalar_tensor_tensor computes qt = min(exp,1)+relu = (elu+1)^T.
    - K path: ek=Exp(K) (ACT), rk=max(K,0) (DVE ts), kf=STT min/add, bf16.
    - vm = [V*mask | mask] bf16 (mask mult on gpsimd, mask col on DVE).
    - 32 matmuls accumulate [KV | Ksum] = kf_j^T @ vm_j into PSUM [64,65].
    - bd = bf16 block-diag [[KV,0],[0,KV]]; zsb = [[Ksum],[Ksum]] cols.
    - 16 matmuls ob[:,t,:] = qt_t^T... lhsT=qt_t, rhs=bd -> raw out rows;
      16 tiny matmuls zn = lhsT=qt_t, rhs=zsb -> Z numerators.
    - One reciprocal + one broadcast-mult normalizes the whole pair;
      output stored bf16 (halves output traffic), upcast on host.
"""

import numpy as np

import concourse.bass as bass
import concourse.mybir as mybir
import concourse.tile as tile
from concourse.bass_utils import run_bass_kernel_spmd
from concourse.masks import make_identity

F32 = mybir.dt.float32
BF16 = mybir.dt.bfloat16
AF = mybir.ActivationFunctionType
OP = mybir.AluOpType

N_CORES = 8
PAIRS = 8          # (b,h) pairs per core
S = 4096
D = 64
J = 32             # rows per partition; s = 32*p + j
T = 16             # transpose blocks per pair (2 j's each)


def build_bass() -> bass.Bass:
    from concourse.bacc import Bacc
    nc = Bacc()
    Qh = nc.dram_tensor("Q", [PAIRS, S, D], F32, kind="ExternalInput")
    Kh = nc.dram_tensor("K", [PAIRS, S, D], F32, kind="ExternalInput")
    Vh = nc.dram_tensor("V", [PAIRS, S, D], F32, kind="ExternalInput")
    Mh = nc.dram_tensor("mask", [PAIRS, S], F32, kind="ExternalInput")
    Oh = nc.dram_tensor("O", [PAIRS, S, D], BF16, kind="ExternalOutput")

    # DRAM views per pair: s = 32*p + j  ->  [128, 32, 64], fully contiguous
    Qv = [Qh[p].rearrange("(p j) d -> p j d", p=128) for p in range(PAIRS)]
    Kv = [Kh[p].rearrange("(p j) d -> p j d", p=128) for p in range(PAIRS)]
    Vv = [Vh[p].rearrange("(p j) d -> p j d", p=128) for p in range(PAIRS)]
    Mv = [Mh[p].rearrange("(p j) -> p j", p=128) for p in range(PAIRS)]
    Ov = [Oh[p].rearrange("(p j) d -> p j d", p=128) for p in range(PAIRS)]

    with tile.TileContext(nc) as tc:
        from contextlib import ExitStack
        with ExitStack() as ctx:
            consts = ctx.enter_context(tc.tile_pool(name="consts", bufs=1))
            q_pool = ctx.enter_context(tc.tile_pool(name="q", bufs=3))
            k_pool = ctx.enter_context(tc.tile_pool(name="k", bufs=3))
            v_pool = ctx.enter_context(tc.tile_pool(name="v", bufs=3))
            m_pool = ctx.enter_context(tc.tile_pool(name="m", bufs=3))
            ek_pool = ctx.enter_context(tc.tile_pool(name="ek", bufs=2))
            rk_pool = ctx.enter_context(tc.tile_pool(name="rk", bufs=2))
            km_pool = ctx.enter_context(tc.tile_pool(name="km", bufs=2))
            qm_pool = ctx.enter_context(tc.tile_pool(name="qm", bufs=2))
            kf_pool = ctx.enter_context(tc.tile_pool(name="kf", bufs=2))
            vm_pool = ctx.enter_context(tc.tile_pool(name="vm", bufs=2))
            et_pool = ctx.enter_context(tc.tile_pool(name="et", bufs=2))
            rt_pool = ctx.enter_context(tc.tile_pool(name="rt", bufs=2))
            qt_pool = ctx.enter_context(tc.tile_pool(name="qt", bufs=2))
            bd_pool = ctx.enter_context(tc.tile_pool(name="bd", bufs=2))
            zsb_pool = ctx.enter_context(tc.tile_pool(name="zsb", bufs=2))
            rec_pool = ctx.enter_context(tc.tile_pool(name="rec", bufs=2))
            osb_pool = ctx.enter_context(tc.tile_pool(name="osb", bufs=2))
            # PSUM budget is 8 banks x 2KB: ob 4 + tp 2 + kv 1 + zn 1
            ob_psum = ctx.enter_context(
                tc.tile_pool(name="obps", bufs=1, space="PSUM"))
            tp_psum = ctx.enter_context(
                tc.tile_pool(name="tpps", bufs=2, space="PSUM"))
            kv_psum = ctx.enter_context(
                tc.tile_pool(name="kvps", bufs=1, space="PSUM"))
            zn_psum = ctx.enter_context(
                tc.tile_pool(name="znps", bufs=1, space="PSUM"))

            identity = consts.tile([128, 128], F32)
            make_identity(nc, identity)

            for p in range(PAIRS):
                q = q_pool.tile([128, J, D], F32)
                k = k_pool.tile([128, J, D], F32)
                v = v_pool.tile([128, J, D], F32)
                m = m_pool.tile([128, J, 1], F32)
                nc.sync.dma_start(out=q, in_=Qv[p])
                nc.sync.dma_start(out=k, in_=Kv[p])
                nc.sync.dma_start(out=v, in_=Vv[p])
                nc.scalar.dma_start(out=m[:, :, 0], in_=Mv[p])

                # ---- Q path: PE-transpose raw fp32 Q, featurize after ----
                tps = []
                for g in range(4):  # 4 tp tiles x 4 transposes each
                    tp = tp_psum.tile([128, 4, 128], F32, tag="tp",
                                      name=f"tp_{p}_{g}")
                    tps.append(tp)
                    for u in range(4):
                        t = 4 * g + u
                        nc.tensor.transpose(
                            tp[:, u, :],
                            q[:, 2 * t:2 * t + 2, :].rearrange(
                                "p a d -> p (a d)"),
                            identity)

                # ---- K path ----
                # kf = min(exp(K),1) + relu(K)  (== elu(K)+1, exactly)
                # exp on ACT, relu on gpsimd, min (4x bf16) + add (2x bf16)
                # on DVE.
                ek = ek_pool.tile([128, J, D], BF16)
                rk = rk_pool.tile([128, J, D], BF16)
                km = km_pool.tile([128, J, D], BF16)
                kf = kf_pool.tile([128, J, D], BF16)
                nc.scalar.activation(ek, k, AF.Exp)
                nc.gpsimd.tensor_scalar_max(rk, k, 0.0)
                nc.vector.tensor_scalar_min(km, ek, 1.0)
                nc.vector.tensor_add(kf, km, rk)

                # ---- V*mask and mask column (gpsimd + DVE) ----
                vm = vm_pool.tile([128, J, D + 1], BF16)
                mb = m[:, :, 0:1].to_broadcast([128, J, D])
                nc.gpsimd.tensor_tensor(
                    out=vm[:, :, 0:D], in0=v, in1=mb, op=OP.mult)
                nc.vector.tensor_copy(vm[:, :, D], m[:, :, 0])

                # ---- Q featurize: ACT exp/relu of tp, fused DVE STT ----
                et = et_pool.tile([128, T, 128], BF16)
                rt = rt_pool.tile([128, T, 128], BF16)
                for g in range(4):
                    src = tps[g].rearrange("p a d -> p (a d)")
                    dst_e = et[:, 4 * g:4 * g + 4, :].rearrange(
                        "p a d -> p (a d)")
                    dst_r = rt[:, 4 * g:4 * g + 4, :].rearrange(
                        "p a d -> p (a d)")
                    nc.scalar.activation(dst_e, src, AF.Exp)
                    nc.scalar.activation(dst_r, src, AF.Relu)
                qm = qm_pool.tile([128, T, 128], BF16)
                qt = qt_pool.tile([128, T, 128], BF16)
                nc.vector.tensor_scalar_min(qm, et, 1.0)
                nc.vector.tensor_add(qt, qm, rt)

                # ---- phase A: [KV | Ksum] accumulation ----
                kvpad = kv_psum.tile([64, 512], F32, tag="kv",
                                     name=f"kv_{p}")
                kvks = kvpad[:, 0:D + 1]
                for j in range(J):
                    nc.tensor.matmul(
                        kvks, lhsT=kf[:, j, :], rhs=vm[:, j, :],
                        start=(j == 0), stop=(j == J - 1))

                # ---- bd/zsb: bf16 block-diag [[KV,0],[0,KV]], Ksum cols ----
                bd = bd_pool.tile([128, 128], BF16)
                zsb = zsb_pool.tile([128, 2], BF16)
                nc.vector.memset(bd, 0.0)
                nc.vector.tensor_copy(bd[0:64, 0:64], kvks[:, 0:64])
                nc.vector.tensor_copy(bd[64:128, 64:128], kvks[:, 0:64])
                nc.vector.memset(zsb, 0.0)
                nc.vector.tensor_copy(zsb[0:64, 0:1], kvks[:, 64:65])
                nc.vector.tensor_copy(zsb[64:128, 1:2], kvks[:, 64:65])

                # ---- phase B: out rows + Z numerators ----
                ob = ob_psum.tile([128, T, 128], F32, tag="ob",
                                  name=f"ob_{p}")
                znpad = zn_psum.tile([128, T, 8], F32, tag="zn",
                                     name=f"zn_{p}")
                zn = znpad[:, :, 0:2]
                for t in range(T):
                    nc.tensor.matmul(ob[:, t, :], lhsT=qt[:, t, :], rhs=bd,
                                     start=True, stop=True)
                    nc.tensor.matmul(zn[:, t, :], lhsT=qt[:, t, :], rhs=zsb,
                                     start=True, stop=True)

                # ---- normalize: one reciprocal + one broadcast-mult ----
                rec = rec_pool.tile([128, T, 2, 1], F32)
                nc.vector.reciprocal(rec[:, :, :, 0], zn)
                osb = osb_pool.tile([128, J, D], BF16)
                nc.vector.tensor_tensor(
                    out=osb.rearrange("p (t u) d -> p t u d", t=T),
                    in0=ob.rearrange("p t (u d) -> p t u d", u=2),
                    in1=rec.to_broadcast([128, T, 2, D]),
                    op=OP.mult)
                nc.scalar.dma_start(out=Ov[p], in_=osb)
    nc.finalize()
    return nc


_NC_CACHE = None


def _get_nc():
    global _NC_CACHE
    if _NC_CACHE is None:
        _NC_CACHE = build_bass()
    return _NC_CACHE


def kernel(Q: np.ndarray, K: np.ndarray, V: np.ndarray, mask: np.ndarray,
           _trace: bool = False):
    B, H = 4, 16
    NP = B * H
    per = NP // N_CORES
    Qr = np.ascontiguousarray(np.asarray(Q, dtype=np.float32).reshape(NP, S, D))
    Kr = np.ascontiguousarray(np.asarray(K, dtype=np.float32).reshape(NP, S, D))
    Vr = np.ascontiguousarray(np.asarray(V, dtype=np.float32).reshape(NP, S, D))
    Mr = np.ascontiguousarray(np.asarray(mask, dtype=np.float32).reshape(NP, S))

    in_maps = []
    for i in range(N_CORES):
        sl = slice(i * per, (i + 1) * per)
        in_maps.append({
            "Q": np.ascontiguousarray(Qr[sl]),
            "K": np.ascontiguousarray(Kr[sl]),
            "V": np.ascontiguousarray(Vr[sl]),
            "mask": np.ascontiguousarray(Mr[sl]),
        })

    nc = _get_nc()
    res = run_bass_kernel_spmd(nc, in_maps, core_ids=list(range(N_CORES)),
                               trace=_trace)
    out = np.concatenate(
        [np.asarray(r["O"]).astype(np.float32) for r in res.results], axis=0)
    if _trace:
        kernel._last_results = res
    return out.reshape(B, H, S, D)


# revision 14
# speedup vs baseline: 2.7959x; 2.7959x over previous
"""Linear attention (elu(x)+1 feature map) Bass/Tile kernel for Trainium2.

Problem: B=4, H=16, S=4096, D=64, fp32.
  Qf = elu(Q)+1; Kf = (elu(K)+1)*mask
  KV = einsum('bhsd,bhse->bhde', Kf, V); Ksum = sum_s Kf*mask
  out = (Qf @ KV) / (Qf . Ksum)

Sharding: 64 (b,h) pairs data-parallel over 8 cores, 8 pairs each. No
collectives.

Per-core v2 design (DMA-bandwidth bound):
  Layout s = 32*p + j (p = SBUF partition, j = 0..31): every Q/K/V/O
  transfer is a single fully-contiguous 1 MB DMA with 8 KB per-partition
  lines (vs 256B lines in v1).
  Per pair:
    - PE transposes raw fp32 Q in 16 [128,128] blocks -> tp PSUM.
    - Scalar ACT computes Exp/Relu of tp into bf16 SBUF; one fused DVE
      scalar_tensor_tensor computes qt = min(exp,1)+relu = (elu+1)^T.
    - K path: ek=Exp(K) (ACT), rk=max(K,0) (DVE ts), kf=STT min/add, bf16.
    - vm = [V*mask | mask] bf16 (mask mult on gpsimd, mask col on DVE).
    - 32 matmuls accumulate [KV | Ksum] = kf_j^T @ vm_j into PSUM [64,65].
    - bd = bf16 block-diag [[KV,0],[0,KV]]; zsb = [[Ksum],[Ksum]] cols.
    - 16 matmuls ob[:,t,:] = qt_t^T... lhsT=qt_t, rhs=bd -> raw out rows;
      16 tiny matmuls zn = lhsT=qt_t, rhs=zsb -> Z numerators.
    - One reciprocal + one broadcast-mult normalizes the whole pair;
      output stored bf16 (halves output traffic), upcast on host.
"""

import numpy as np

import concourse.bass as bass
import concourse.mybir as mybir
import concourse.tile as tile
from concourse.bass_utils import run_bass_kernel_spmd
from concourse.masks import make_identity

F32 = mybir.dt.float32
BF16 = mybir.dt.bfloat16
AF = mybir.ActivationFunctionType
OP = mybir.AluOpType

N_CORES = 8
PAIRS = 8          # (b,h) pairs per core
S = 4096
D = 64
J = 32             # rows per partition; s = 32*p + j
T = 16             # transpose blocks per pair (2 j's each)


def build_bass() -> bass.Bass:
    from concourse.bacc import Bacc
    nc = Bacc()
    Qh = nc.dram_tensor("Q", [PAIRS, S, D], F32, kind="ExternalInput")
    Kh = nc.dram_tensor("K", [PAIRS, S, D], F32, kind="ExternalInput")
    Vh = nc.dram_tensor("V", [PAIRS, S, D], F32, kind="ExternalInput")
    Mh = nc.dram_tensor("mask", [PAIRS, S], F32, kind="ExternalInput")
    Oh = nc.dram_tensor("O", [PAIRS, S, D], BF16, kind="ExternalOutput")

    # DRAM views per pair: s = 32*p + j  ->  [128, 32, 64], fully contiguous
    Qv = [Qh[p].rearrange("(p j) d -> p j d", p=128) for p in range(PAIRS)]
    Kv = [Kh[p].rearrange("(p j) d -> p j d", p=128) for p in range(PAIRS)]
    Vv = [Vh[p].rearrange("(p j) d -> p j d", p=128) for p in range(PAIRS)]
    Mv = [Mh[p].rearrange("(p j) -> p j", p=128) for p in range(PAIRS)]
    Ov = [Oh[p].rearrange("(p j) d -> p j d", p=128) for p in range(PAIRS)]

    with tile.TileContext(nc) as tc:
        from contextlib import ExitStack
        with ExitStack() as ctx:
            consts = ctx.enter_context(tc.tile_pool(name="consts", bufs=1))
            q_pool = ctx.enter_context(tc.tile_pool(name="q", bufs=3))
            k_pool = ctx.enter_context(tc.tile_pool(name="k", bufs=3))
            v_pool = ctx.enter_context(tc.tile_pool(name="v", bufs=3))
            m_pool = ctx.enter_context(tc.tile_pool(name="m", bufs=3))
            ek_pool = ctx.enter_context(tc.tile_pool(name="ek", bufs=2))
            rk_pool = ctx.enter_context(tc.tile_pool(name="rk", bufs=2))
            km_pool = ctx.enter_context(tc.tile_pool(name="km", bufs=2))
            qm_pool = ctx.enter_context(tc.tile_pool(name="qm", bufs=2))
            kf_pool = ctx.enter_context(tc.tile_pool(name="kf", bufs=2))
            vm_pool = ctx.enter_context(tc.tile_pool(name="vm", bufs=2))
            et_pool = ctx.enter_context(tc.tile_pool(name="et", bufs=2))
            rt_pool = ctx.enter_context(tc.tile_pool(name="rt", bufs=2))
            qt_pool = ctx.enter_context(tc.tile_pool(name="qt", bufs=2))
            bd_pool = ctx.enter_context(tc.tile_pool(name="bd", bufs=2))
            zsb_pool = ctx.enter_context(tc.tile_pool(name="zsb", bufs=2))
            rec_pool = ctx.enter_context(tc.tile_pool(name="rec", bufs=2))
            osb_pool = ctx.enter_context(tc.tile_pool(name="osb", bufs=2))
            # PSUM budget is 8 banks x 2KB: ob 4 + tp 2 + kv 1 + zn 1
            ob_psum = ctx.enter_context(
                tc.tile_pool(name="obps", bufs=1, space="PSUM"))
            tp_psum = ctx.enter_context(
                tc.tile_pool(name="tpps", bufs=2, space="PSUM"))
            kv_psum = ctx.enter_context(
                tc.tile_pool(name="kvps", bufs=1, space="PSUM"))
            zn_psum = ctx.enter_context(
                tc.tile_pool(name="znps", bufs=1, space="PSUM"))

            identity = consts.tile([128, 128], F32)
            make_identity(nc, identity)

            for p in range(PAIRS):
                q = q_pool.tile([128, J, D], F32)
                k = k_pool.tile([128, J, D], F32)
                v = v_pool.tile([128, J, D], F32)
                m = m_pool.tile([128, J, 1], F32)
                nc.sync.dma_start(out=q, in_=Qv[p])
                nc.sync.dma_start(out=k, in_=Kv[p])
                nc.sync.dma_start(out=v, in_=Vv[p])
                nc.scalar.dma_start(out=m[:, :, 0], in_=Mv[p])

                # ---- Q path: PE-transpose raw fp32 Q, featurize after ----
                tps = []
                for g in range(4):  # 4 tp tiles x 4 transposes each
                    tp = tp_psum.tile([128, 4, 128], F32, tag="tp",
                                      name=f"tp_{p}_{g}")
                    tps.append(tp)
                    for u in range(4):
                        t = 4 * g + u
                        nc.tensor.transpose(
                            tp[:, u, :],
                            q[:, 2 * t:2 * t + 2, :].rearrange(
                                "p a d -> p (a d)"),
                            identity)

                # ---- K path ----
                # kf = min(exp(K),1) + relu(K)  (== elu(K)+1, exactly)
                # exp on ACT, relu on gpsimd, min (4x bf16) + add (2x bf16)
                # on DVE.
                ek = ek_pool.tile([128, J, D], BF16)
                rk = rk_pool.tile([128, J, D], BF16)
                km = km_pool.tile([128, J, D], BF16)
                kf = kf_pool.tile([128, J, D], BF16)
                nc.scalar.activation(ek, k, AF.Exp)
                nc.vector.tensor_scalar_max(rk, k, 0.0)
                nc.vector.tensor_scalar_min(km, ek, 1.0)
                nc.vector.tensor_add(kf, km, rk)

                # ---- V*mask and mask column (gpsimd + DVE) ----
                vm = vm_pool.tile([128, J, D + 1], BF16)
                mb = m[:, :, 0:1].to_broadcast([128, J, D])
                nc.gpsimd.tensor_tensor(
                    out=vm[:, :, 0:D], in0=v, in1=mb, op=OP.mult)
                nc.vector.tensor_copy(vm[:, :, D], m[:, :, 0])

                # ---- Q featurize: ACT exp/relu of tp, fused DVE STT ----
                et = et_pool.tile([128, T, 128], BF16)
                rt = rt_pool.tile([128, T, 128], BF16)
                for g in range(4):
                    src = tps[g].rearrange("p a d -> p (a d)")
                    dst_e = et[:, 4 * g:4 * g + 4, :].rearrange(
                        "p a d -> p (a d)")
                    dst_r = rt[:, 4 * g:4 * g + 4, :].rearrange(
                        "p a d -> p (a d)")
                    nc.scalar.activation(dst_e, src, AF.Exp)
                    nc.scalar.activation(dst_r, src, AF.Relu)
                qm = qm_pool.tile([128, T, 128], BF16)
                qt = qt_pool.tile([128, T, 128], BF16)
                nc.vector.tensor_scalar_min(qm, et, 1.0)
                nc.vector.tensor_add(qt, qm, rt)

                # ---- phase A: [KV | Ksum] accumulation ----
                kvpad = kv_psum.tile([64, 512], F32, tag="kv",
                                     name=f"kv_{p}")
                kvks = kvpad[:, 0:D + 1]
                for j in range(J):
                    nc.tensor.matmul(
                        kvks, lhsT=kf[:, j, :], rhs=vm[:, j, :],
                        start=(j == 0), stop=(j == J - 1))

                # ---- bd/zsb: bf16 block-diag [[KV,0],[0,KV]], Ksum cols ----
                bd = bd_pool.tile([128, 128], BF16)
                zsb = zsb_pool.tile([128, 2], BF16)
                nc.vector.memset(bd, 0.0)
                nc.vector.tensor_copy(bd[0:64, 0:64], kvks[:, 0:64])
                nc.vector.tensor_copy(bd[64:128, 64:128], kvks[:, 0:64])
                nc.vector.memset(zsb, 0.0)
                nc.vector.tensor_copy(zsb[0:64, 0:1], kvks[:, 64:65])
                nc.vector.tensor_copy(zsb[64:128, 1:2], kvks[:, 64:65])

                # ---- phase B: out rows + Z numerators ----
                ob = ob_psum.tile([128, T, 128], F32, tag="ob",
                                  name=f"ob_{p}")
                znpad = zn_psum.tile([128, T, 8], F32, tag="zn",
                                     name=f"zn_{p}")
                zn = znpad[:, :, 0:2]
                for t in range(T):
                    nc.tensor.matmul(ob[:, t, :], lhsT=qt[:, t, :], rhs=bd,
                                     start=True, stop=True)
                    nc.tensor.matmul(zn[:, t, :], lhsT=qt[:, t, :], rhs=zsb,
                                     start=True, stop=True)

                # ---- normalize: one reciprocal + one broadcast-mult ----
                rec = rec_pool.tile([128, T, 2, 1], F32)
                nc.vector.reciprocal(rec[:, :, :, 0], zn)
                osb = osb_pool.tile([128, J, D], BF16)
                nc.vector.tensor_tensor(
                    out=osb.rearrange("p (t u) d -> p t u d", t=T),
                    in0=ob.rearrange("p t (u d) -> p t u d", u=2),
                    in1=rec.to_broadcast([128, T, 2, D]),
                    op=OP.mult)
                nc.scalar.dma_start(out=Ov[p], in_=osb)
    nc.finalize()
    return nc


_NC_CACHE = None


def _get_nc():
    global _NC_CACHE
    if _NC_CACHE is None:
        _NC_CACHE = build_bass()
    return _NC_CACHE


def kernel(Q: np.ndarray, K: np.ndarray, V: np.ndarray, mask: np.ndarray,
           _trace: bool = False):
    B, H = 4, 16
    NP = B * H
    per = NP // N_CORES
    Qr = np.ascontiguousarray(np.asarray(Q, dtype=np.float32).reshape(NP, S, D))
    Kr = np.ascontiguousarray(np.asarray(K, dtype=np.float32).reshape(NP, S, D))
    Vr = np.ascontiguousarray(np.asarray(V, dtype=np.float32).reshape(NP, S, D))
    Mr = np.ascontiguousarray(np.asarray(mask, dtype=np.float32).reshape(NP, S))

    in_maps = []
    for i in range(N_CORES):
        sl = slice(i * per, (i + 1) * per)
        in_maps.append({
            "Q": np.ascontiguousarray(Qr[sl]),
            "K": np.ascontiguousarray(Kr[sl]),
            "V": np.ascontiguousarray(Vr[sl]),
            "mask": np.ascontiguousarray(Mr[sl]),
        })

    nc = _get_nc()
    res = run_bass_kernel_spmd(nc, in_maps, core_ids=list(range(N_CORES)),
                               trace=_trace)
    out = np.concatenate(
        [np.asarray(r["O"]).astype(np.float32) for r in res.results], axis=0)
    if _trace:
        kernel._last_results = res
    return out.reshape(B, H, S, D)
